# revision 2
# baseline (speedup 1.0000x reference)
"""ROIPool (adaptive max pool over ROI crops) for Trainium2, 8-core SPMD.

Strategy:
  - Host computes every ROI's crop geometry and adaptive-pool bin boundaries
    from the actual inputs (floor/ceil in float32, exactly as the reference).
  - ROIs are sorted by source image and dealt to the 8 cores in contiguous
    chunks, so each core touches only 1-2 source images.
  - One SPMD Bass program is compiled per call.  Per-core work is baked as
    static instruction streams behind tc.If(partition_id == k) branches:
    each core DMAs its source image(s) into SBUF once ([128 partitions, 2
    channel-halves, 56*56]) and then runs exact statically-sliced
    reduce_max instructions per ROI:
       row phase: for each run of row-bins with equal size+stride, one DVE
                  reduce over the h axis -> rowp[q, i, w_rel]
       col phase: for each run of col-bins, one DVE reduce over w -> out
    Outputs are staged 8 ROIs at a time and DMA'd to DRAM.
  - Max in f32 is exact, so the result is bitwise identical to the
    reference regardless of reduction order.
"""

import numpy as np

OUT = 7
NCORES = 8


# ---------------------------------------------------------------- host math
def _roi_geometry(images_shape, rois, roi_idx):
    """Replicates the reference's f32 floor/ceil box math on the host."""
    n, c, h, w = images_shape
    rois = np.asarray(rois, dtype=np.float32)
    roi_idx = np.asarray(roi_idx).astype(np.int64)
    x1 = np.floor(rois[:, 0] * np.float32(w)).astype(np.int64)
    y1 = np.floor(rois[:, 1] * np.float32(h)).astype(np.int64)
    x2 = np.ceil(rois[:, 2] * np.float32(w)).astype(np.int64)
    y2 = np.ceil(rois[:, 3] * np.float32(h)).astype(np.int64)
    return x1, y1, x2, y2, roi_idx


def _bins(start, length):
    """Adaptive pool bins [start + i*L//OUT, start + ceil((i+1)*L/OUT)) ."""
    out = []
    for i in range(OUT):
        s = start + (i * length) // OUT
        e = start + ((i + 1) * length + OUT - 1) // OUT
        out.append((int(s), int(e)))
    return out


def _runs(bins):
    """Group consecutive bins into maximal runs with equal size and equal
    start stride.  Returns list of (i0, count, s0, stride, size)."""
    runs = []
    i = 0
    while i < OUT:
        s0, e0 = bins[i]
        size = e0 - s0
        j = i + 1
        stride = None
        while j < OUT:
            s, e = bins[j]
            if e - s != size:
                break
            st = s - bins[j - 1][0]
            if stride is None:
                stride = st
            elif st != stride:
                break
            j += 1
        runs.append((i, j - i, s0, 0 if stride is None else stride, size))
        i = j
    return runs


# ---------------------------------------------------------------- kernel
def kernel(images, rois, roi_idx):
    import concourse.bacc as bacc
    import concourse.bass as bass
    import concourse.mybir as mybir
    from concourse.bass_utils import run_bass_kernel_spmd
    from concourse.tile import TileContext

    images = np.ascontiguousarray(np.asarray(images, dtype=np.float32))
    N, C, H, W = images.shape
    R = np.asarray(rois).shape[0]
    assert C % 256 == 0 and C == 256 and H == 56 and W == 56, (
        "kernel hardcoded for C=256, H=W=56"
    )
    HW = H * W
    x1, y1, x2, y2, ridx = _roi_geometry(images.shape, rois, roi_idx)

    # order ROIs by source image; contiguous chunks per core
    order = np.argsort(ridx, kind="stable")
    chunks = np.array_split(order, NCORES)
    nr_max = max(len(ch) for ch in chunks)
    oct_max = (nr_max + 7) // 8

    nc = bacc.Bacc("TRN2", target_bir_lowering=False, debug=False, num_devices=NCORES)
    images_d = nc.dram_tensor("images", [N, C, H, W], mybir.dt.float32, kind="ExternalInput")
    out_d = nc.dram_tensor("out", [nr_max, C, OUT, OUT], mybir.dt.float32, kind="ExternalOutput")

    NEG = float(np.finfo(np.float32).min)

    with TileContext(nc) as tc:
        pid = nc.partition_id()
        with (
            tc.tile_pool(name="img", bufs=1) as img_pool,
            tc.tile_pool(name="wrk", bufs=1) as wrk_pool,
        ):
            # image tiles: [128, 2, HW]; c = q*128 + p
            G_ALLOC = 4
            img_tiles = [
                img_pool.tile([128, 2, HW], mybir.dt.float32, tag=f"img{g}", name=f"img{g}")
                for g in range(G_ALLOC)
            ]
            rowp = wrk_pool.tile([128, 2, OUT, W], mybir.dt.float32, tag="rowp")
            stags = [
                wrk_pool.tile([128, 8, 2, OUT * OUT], mybir.dt.float32, tag=f"stag{b}", name=f"stag{b}")
                for b in range(2)
            ]
            IMG_P = 2 * HW          # img tile partition pitch (elements)
            ROW_P = 2 * OUT * W     # rowp partition pitch
            STAG_P = 8 * 2 * OUT * OUT

            for k in range(NCORES):
                chunk = chunks[k]
                if len(chunk) == 0:
                    continue
                with tc.If(pid == k):
                    # distinct images for this core, in first-use order
                    groups = []
                    g_of = {}
                    for r in chunk:
                        n = int(ridx[r])
                        if n not in g_of:
                            g_of[n] = len(groups)
                            groups.append(n)
                    for g, n in enumerate(groups):
                        src = images_d[n].rearrange("(q p) h w -> p q (h w)", q=2)
                        nc.sync.dma_start(out=img_tiles[g % G_ALLOC][:], in_=src)

                    for idx, r in enumerate(chunk):
                        g = g_of[int(ridx[r])] % G_ALLOC
                        it = img_tiles[g]
                        X1, Y1, X2, Y2 = int(x1[r]), int(y1[r]), int(x2[r]), int(y2[r])
                        Hr, Wr = Y2 - Y1, X2 - X1
                        r8 = idx % 8
                        stag = stags[(idx // 8) % 2]
                        if Hr <= 0 or Wr <= 0:
                            # degenerate box: reference yields finfo.min
                            nc.vector.memset(stag[:, r8], NEG)
                        else:
                            # ---- row phase: img -> rowp[q, i, 0:Wr]
                            for (i0, cnt, s0, stride, size) in _runs(_bins(Y1, Hr)):
                                in_ap = [(IMG_P, 128), (HW, 2)]
                                out_ap = [(ROW_P, 128), (OUT * W, 2)]
                                if cnt > 1:
                                    in_ap.append((stride * W, cnt))
                                    out_ap.append((W, cnt))
                                in_ap.append((1, Wr))
                                out_ap.append((1, Wr))
                                in_ap.append((W, size))
                                nc.vector.tensor_reduce(
                                    bass.AP(rowp.tensor, rowp[:].offset + i0 * W, out_ap),
                                    bass.AP(it.tensor, it[:].offset + s0 * W + X1, in_ap),
                                    axis=mybir.AxisListType.X,
                                    op=mybir.AluOpType.max,
                                )
                            # ---- col phase: rowp -> stag[r8][q, i, j]
                            for (j0, cnt, c0, stride, size) in _runs(_bins(0, Wr)):
                                in_ap = [(ROW_P, 128), (OUT * W, 2)]
                                out_ap = [(STAG_P, 128), (OUT * OUT, 2)]
                                if cnt > 1:
                                    in_ap.append((stride, cnt))
                                    out_ap.append((1, cnt))
                                in_ap.append((W, OUT))
                                out_ap.append((OUT, OUT))
                                in_ap.append((1, size))
                                nc.vector.tensor_reduce(
                                    bass.AP(
                                        stag.tensor,
                                        stag[:].offset + r8 * 2 * OUT * OUT + j0,
                                        out_ap,
                                    ),
                                    bass.AP(rowp.tensor, rowp[:].offset + c0, in_ap),
                                    axis=mybir.AxisListType.X,
                                    op=mybir.AluOpType.max,
                                )
                        # flush every 8 ROIs (or at end)
                        if r8 == 7 or idx == len(chunk) - 1:
                            cnt_r = r8 + 1
                            r0 = idx - r8
                            dst = out_d[r0 : r0 + cnt_r].rearrange(
                                "r (q p) i j -> p r q (i j)", q=2
                            )
                            nc.sync.dma_start(out=dst, in_=stag[:, 0:cnt_r])

    nc.compile()

    in_maps = [{"images": images} for _ in range(NCORES)]
    res = run_bass_kernel_spmd(nc, in_maps, list(range(NCORES)))

    full = np.empty((R, C, OUT, OUT), dtype=np.float32)
    for k in range(NCORES):
        ch = chunks[k]
        if len(ch):
            full[ch] = res.results[k]["out"][: len(ch)]
    return full
